# revision 34
# baseline (speedup 1.0000x reference)
"""Trainium2 Bass kernel for the 4-layer adaLN causal transformer (v3.2).

v3.2 = v3 + fp8e4m3 AllGather payload, consumed directly by the PE:
k and v are staged to the collective in fp8 (1 byte), halving the gathered
bytes (4.7MB -> 2.36MB out per layer), and the gathered fp8 tiles feed the
attention matmuls directly as lhsT (mixed fp8 x f16/bf16 matmuls -- no
conversion pass; results are bit-identical to explicit DVE conversion).
k is scaled by KSC=8 into fp8 range (bias pre-scaled on host, exp scale
divided by KSC); v and the denominator ones-column are both scaled by KSC so
the softmax ratio is exact. KSC must keep |KSC*(k+bias)| well under 448 --
e4m3 saturates to NaN (KSC=16 overflowed; max |k+bias| ~ 22). Measured
rel err 1.35e-2 (budget 2e-2); paired A/B slope ~ -13% vs v3.

v3: attention restructured for instruction efficiency: e-matmuls write wide
PSUM groups ([128,1024]/[128,640] per head), ONE exp per group, bf16
kill-mask multiplies (causal block-kill + diag triangle) on DVE/Pool, key
padding folded into zeroed v rows/ones at staging, softmax denominators
processed per head-pair.

Sharding: sequence-parallel. 8 cores = 2 batch groups x 4 token shards.
Core (b, s) owns query blocks {s, 7-s} of batch b (zigzag for causal load
balance; every core sees exactly 9 key-blocks of true attention work).
Weights are replicated (full, adaLN-folded per batch, fp16); there are NO
AllReduces. Per layer the only collectives are two small AllGathers (k and
v across the 4 shards), and the diagonal attention blocks use local k/v so
they start before the AllGather lands.

Activations stay transposed xT[d, t_local] (f32r residual stream, fp16
modulated hT). The adaLN affine is folded into Wqkv/W1 on the host exactly
as in the TP kernel: gamma scales weight rows, beta enters as per-partition
biases on q/k and the gelu, and the v-beta contribution folds into bout.
Attention avoids max-subtraction: exp() rides the ACT bias with -1e30 key
masks; aT/v are bf16 (range) while everything else is fp16.

Residuals are masked every sub-block ((x+f)*m, matching the reference), so
padded-token columns stay exactly 0 and nothing can overflow -> no clamps.

Uniform SPMD program: per-core differences live only in input data
(madd2/mdiag mask tables, xT slices); off-diagonal passes that a core does
not need are killed by -1e30 masks (~25% wasted attention work).
"""

import os
import numpy as np

import concourse.bacc as bacc
import concourse.mybir as mybir
from concourse.tile import TileContext
from concourse.bass_utils import run_bass_kernel_spmd

F32 = mybir.dt.float32
F32R = mybir.dt.float32r
F16 = mybir.dt.float16
BF16 = mybir.dt.bfloat16
FP8 = mybir.dt.float8e4
AF = mybir.ActivationFunctionType
ALU = mybir.AluOpType

D = 1024
T = 1024
L = 4
CH = 256            # local tokens per core (2 blocks of 128)
KC = 8              # d_model chunks
ADALN_K = 0.1
EPS = 1e-5
KSC = 8.0           # k/v pre-scale into fp8e4m3 range for the AllGather
                    # (must keep KSC*(k+bias) well under 448: e4m3 saturates to NaN)
SCALE = 0.125 / KSC
RG = [[0, 1, 2, 3], [4, 5, 6, 7]]
NJ0 = 3             # uniform off-diag key blocks for iq=0 (covers qb=s<=3)
NJ1 = 7             # for iq=1 (covers qb=7-s<=7)
REPS = int(os.environ.get("BK2_REPS", "1"))
SKIP_AG = bool(int(os.environ.get("BK2_SKIP_AG", "0")))    # timing ablation
SKIP_ATTN = bool(int(os.environ.get("BK2_SKIP_ATTN", "0")))
SKIP_FFN = bool(int(os.environ.get("BK2_SKIP_FFN", "0")))
WONCE = bool(int(os.environ.get("BK2_WONCE", "0")))        # timing ablation: 1 DMA per weight kind/layer
EXP_DVE = bool(int(os.environ.get("BK2_EXP_DVE", "0")))    # timing ablation: exp -> DVE copy

_CACHED = {}


def _build_nc():
    nc = bacc.Bacc(target_bir_lowering=False, debug=False)

    xT_d = nc.dram_tensor("xT", [D, CH], F32, kind="ExternalInput")
    # weights pre-rearranged on host: [L, mb, kc, p, c] so strips DMA contiguously
    wqk_d = nc.dram_tensor("wqk", [L, 16, 128, KC, 128], F16, kind="ExternalInput")
    wv_d = nc.dram_tensor("wv", [L, 128, KC, D], F16, kind="ExternalInput")
    wout_d = nc.dram_tensor("wout", [L, 8, 128, KC, 128], F16, kind="ExternalInput")
    w1_d = nc.dram_tensor("w1", [L, 32, 128, KC, 128], F16, kind="ExternalInput")
    w2_d = nc.dram_tensor("w2", [L, 8, 128, 32, 128], F16, kind="ExternalInput")
    qkb_d = nc.dram_tensor("qkb", [L, 128, 16], F32, kind="ExternalInput")
    b1t_d = nc.dram_tensor("b1t", [L, 128, 32], F32, kind="ExternalInput")
    bsum_d = nc.dram_tensor("bsum", [L, 2, 128, 8], F32, kind="ExternalInput")
    kmask_d = nc.dram_tensor("kmask", [128, 1664], BF16, kind="ExternalInput")
    mqp_d = nc.dram_tensor("mqp", [128, 2], F32, kind="ExternalInput")
    mrow_d = nc.dram_tensor("mrow", [1, CH], F32, kind="ExternalInput")
    ones_d = nc.dram_tensor("onescol", [128, 1], F32, kind="ExternalInput")
    kb_d = nc.dram_tensor("kbias", [128, 1], F32, kind="ExternalInput")
    out_d = nc.dram_tensor("out_xT", [D, CH], F32, kind="ExternalOutput")

    with TileContext(nc) as tc:
        with nc.allow_low_precision("fp16/bf16 intermediates by design"), \
             tc.tile_pool(name="pers", bufs=1) as pers, \
             tc.tile_pool(name="wp", bufs=5) as wp, \
             tc.tile_pool(name="wf2", bufs=2) as wf2, \
             tc.tile_pool(name="cst", bufs=8) as cst, \
             tc.tile_pool(name="tp", bufs=3) as tp, \
             tc.tile_pool(name="tp4", bufs=6) as tp4, \
             tc.tile_pool(name="wvp", bufs=1) as wvp, \
             tc.tile_pool(name="ps", bufs=2, space="PSUM") as ps, \
             tc.tile_pool(name="pse", bufs=2, space="PSUM") as pse, \
             tc.tile_pool(name="pso", bufs=2, space="PSUM") as pso, \
             tc.tile_pool(name="dr", bufs=2, space="DRAM") as dr:

            # ---- persistent tiles ----
            xT = pers.tile([128, KC * CH], F32R, tag="xT")
            hT = pers.tile([128, KC * CH], F16, tag="hT")
            qT = pers.tile([128, 8 * CH], F16, tag="qT")       # pair m at m*CH, iq block at +iq*128
            kst = pers.tile([128, 8 * CH], F16, tag="kst")     # local k*KSC (also diag lhsT)
            kst8 = pers.tile([128, 8 * CH], FP8, tag="kst8")   # fp8 AG payload (k)
            kT8 = pers.tile([128, 64 * 128], FP8, tag="kT8")   # gathered k (fp8), (m,jb) at (m*8+jb)*128
            vstg = pers.tile([128, 2 * 1280], BF16, tag="vstg")  # local v*KSC+ones (65/head, pad 1280)
            vstg8 = pers.tile([128, 2 * 1280], FP8, tag="vstg8")  # fp8 AG payload (v)
            vst8 = pers.tile([128, 8 * 1280], FP8, tag="vst8")    # gathered v (fp8): (jb, h) at jb*1280+h*65
            oT = pers.tile([128, KC * CH], F16, tag="oT")
            ffT = pers.tile([128, 32 * CH], F16, tag="ffT")
            onesK = pers.tile([128, 1], F32R, tag="onesK")
            onesB = pers.tile([1, 128], F32R, tag="onesB")
            km_b = pers.tile([128, 1664], BF16, tag="kmb")
            kb_t = pers.tile([128, 1], F32, tag="kb")
            mqp_t = pers.tile([128, 2], F32, tag="mqp")
            mrow_r = pers.tile([1, CH], F32R, tag="mrow")
            mbT = pers.tile([128, CH], F32, tag="mbT")         # mask broadcast

            nc.sync.dma_start(onesK[:, :], ones_d[:, :].bitcast(F32R))
            nc.sync.dma_start(onesB[:, :], ones_d[:, 0:1].bitcast(F32R).rearrange("p 1 -> 1 p"))
            nc.sync.dma_start(km_b[:, :], kmask_d[:, :])
            nc.sync.dma_start(kb_t[:, :], kb_d[:, :])
            nc.sync.dma_start(mqp_t[:, :], mqp_d[:, :])
            nc.sync.dma_start(mrow_r[:, :], mrow_d[:, :].bitcast(F32R))
            # ones columns for the softmax denominators (x=64 of each 65-strip)
            for blk in range(2):
                ones_ap = (vstg[:, blk * 1280: blk * 1280 + 1040]
                           .rearrange("p (s x) -> p s x", x=65)[:, :, 64:65])
                nc.vector.memset(ones_ap, 1.0)
                nc.vector.tensor_scalar_mul(ones_ap, ones_ap, mqp_t[:, blk:blk + 1])
                nc.vector.memset(vstg[:, blk * 1280 + 1040:(blk + 1) * 1280], 0.0)
            for c in range(KC):
                nc.sync.dma_start(
                    xT[:, c * CH:(c + 1) * CH],
                    xT_d[c * 128:(c + 1) * 128, :].bitcast(F32R))
            # mask broadcast [128, CH]
            pm = ps.tile([128, CH], F32, tag="ps")
            nc.tensor.matmul(pm[:, :], onesB[:, :], mrow_r[:, :], start=True, stop=True)
            nc.vector.tensor_copy(mbT[:, :], pm[:, :])

            consts = {}

            def load_layer_consts(layer):
                qkb = cst.tile([128, 16], F32, tag="qkb")
                nc.sync.dma_start(qkb[:, :], qkb_d[layer])
                b1t = cst.tile([128, 32], F32, tag="b1")
                nc.sync.dma_start(b1t[:, :], b1t_d[layer])
                bs0 = cst.tile([128, 8], F32, tag="bs0")
                nc.sync.dma_start(bs0[:, :], bsum_d[layer, 0])
                bs1 = cst.tile([128, 8], F32, tag="bs1")
                nc.sync.dma_start(bs1[:, :], bsum_d[layer, 1])
                consts.update({"qkb": qkb, "b1": b1t, 0: bs0, 1: bs1})

            wcache = {}

            def wload(w_d, layer, mb, kcnt, pool, tag):
                key = id(w_d)
                if WONCE and key in wcache:
                    return wcache[key]
                wt = pool.tile([128, kcnt * 128], F16, tag=tag)
                nc.sync.dma_start(
                    wt[:, :].rearrange("p (k c) -> p k c", k=kcnt), w_d[layer, mb])
                wcache[key] = wt
                return wt

            def emit_adaln(layer, sb):
                """hT = mod(norm(xT)); affine folded into consuming weights."""
                ps_sum = ps.tile([1, CH], F32, tag="ps")
                ps_sq = ps.tile([1, CH], F32, tag="ps")
                for c in range(KC):
                    xs = xT[:, c * CH:(c + 1) * CH]
                    xsq = tp.tile([128, CH], F32R, tag="xsq")
                    nc.scalar.activation(xsq[:, :], xs, AF.Square)
                    nc.tensor.matmul(ps_sum[:, :], onesK[:, :], xs,
                                     start=(c == 0), stop=(c == KC - 1))
                    nc.tensor.matmul(ps_sq[:, :], onesK[:, :], xsq[:, :],
                                     start=(c == 0), stop=(c == KC - 1))
                murow = tp.tile([1, CH], F32R, tag="murow")
                nc.scalar.mul(murow[:, :], ps_sum[0:1, :], 1.0 / D)
                m2row = tp.tile([1, CH], F32, tag="m2row")
                nc.scalar.mul(m2row[:, :], ps_sq[0:1, :], 1.0 / D)
                musq = tp.tile([1, CH], F32, tag="musq")
                nc.vector.tensor_tensor(musq[:, :], murow[:, :], murow[:, :], ALU.mult)
                nc.vector.tensor_tensor(m2row[:, :], m2row[:, :], musq[:, :], ALU.subtract)
                nc.vector.tensor_scalar_add(m2row[:, :], m2row[:, :], EPS)
                nc.scalar.activation(musq[:, :], m2row[:, :], AF.Sqrt)
                rrow = tp.tile([1, CH], F32R, tag="rrow")
                nc.vector.reciprocal(rrow[:, :], musq[:, :])
                mrs = tp.tile([1, CH], F32R, tag="mrs")
                nc.vector.tensor_tensor(mrs[:, :], murow[:, :], rrow[:, :], ALU.mult)
                ps_rs = ps.tile([128, CH], F32, tag="ps")
                nc.tensor.matmul(ps_rs[:, :], onesB[:, :], rrow[:, :], start=True, stop=True)
                ps_mrs = ps.tile([128, CH], F32, tag="ps")
                nc.tensor.matmul(ps_mrs[:, :], onesB[:, :], mrs[:, :], start=True, stop=True)
                rsb = tp.tile([128, CH], F32, tag="rsb")
                nc.vector.tensor_copy(rsb[:, :], ps_rs[:, :])
                mrsb = tp.tile([128, CH], F32, tag="mrsb")
                nc.vector.tensor_copy(mrsb[:, :], ps_mrs[:, :])
                for c in range(KC):
                    xs = xT[:, c * CH:(c + 1) * CH]
                    t0 = tp.tile([128, CH], F32, tag="t0")
                    eng = nc.vector if c % 2 == 0 else nc.gpsimd
                    eng.tensor_tensor(t0[:, :], xs, rsb[:, :], ALU.mult)
                    eng.tensor_tensor(t0[:, :], t0[:, :], mrsb[:, :], ALU.subtract)
                    nc.scalar.activation(
                        hT[:, c * CH:(c + 1) * CH], t0[:, :],
                        AF.Square, scale=float(ADALN_K ** 0.5), bias=kb_t[:, 0:1])

            def emit_k(layer, ag_in):
                qkb = consts["qkb"]
                for m in range(8):
                    mb = 8 + m
                    wt = wload(wqk_d, layer, mb, KC, wp, "wa")
                    pq = ps.tile([128, CH], F32, tag="ps")
                    for kk in range(KC):
                        nc.tensor.matmul(pq[:, :], wt[:, kk * 128:(kk + 1) * 128],
                                         hT[:, kk * CH:(kk + 1) * CH],
                                         start=(kk == 0), stop=(kk == KC - 1))
                    # kst = KSC*(k + bias): scale into fp8 range (bias pre-scaled on host)
                    nc.scalar.activation(kst[:, m * CH:(m + 1) * CH], pq[:, :],
                                         AF.Identity, scale=KSC, bias=qkb[:, mb:mb + 1])
                    eng = nc.vector if m % 2 == 0 else nc.gpsimd
                    eng.tensor_copy(kst8[:, m * CH:(m + 1) * CH],
                                    kst[:, m * CH:(m + 1) * CH])
                    nc.sync.dma_start(ag_in[m * 128:(m + 1) * 128, :],
                                      kst8[:, m * CH:(m + 1) * CH])

            def load_wv(layer):
                wvL = wvp.tile([128, KC * D], F16, tag="wv")
                nc.sync.dma_start(
                    wvL[:, :].rearrange("p (k c) -> p k c", k=KC), wv_d[layer])
                return wvL

            def emit_v(layer, ag_in, wvL):
                for blk in range(2):
                    for half in range(2):
                        pv = ps.tile([128, 512], F32, tag="ps")
                        for kc in range(KC):
                            nc.tensor.matmul(
                                pv[:, :],
                                hT[:, kc * CH + blk * 128: kc * CH + blk * 128 + 128],
                                wvL[:, kc * D + half * 512: kc * D + (half + 1) * 512],
                                start=(kc == 0), stop=(kc == KC - 1))
                        # v psum [128tok, 512 vd] -> vstg strips, zeroing pad-token rows
                        nc.vector.tensor_scalar_mul(
                            vstg[:, blk * 1280 + half * 8 * 65: blk * 1280 + (half * 8 + 8) * 65]
                            .rearrange("p (h x) -> p h x", x=65)[:, :, 0:64],
                            pv[:, :].rearrange("p (h d) -> p h d", d=64),
                            mqp_t[:, blk:blk + 1])
                    eng = nc.vector if blk == 0 else nc.gpsimd
                    eng.tensor_copy(vstg8[:, blk * 1280:(blk + 1) * 1280],
                                    vstg[:, blk * 1280:(blk + 1) * 1280])
                    nc.sync.dma_start(
                        ag_in[1024 + blk * 640: 1024 + (blk + 1) * 640, :]
                        .rearrange("(p x) c -> p (x c)", p=128),
                        vstg8[:, blk * 1280:(blk + 1) * 1280])

            def emit_q(layer):
                qkb = consts["qkb"]
                for m in range(8):
                    wt = wload(wqk_d, layer, m, KC, wp, "wa")
                    pq = ps.tile([128, CH], F32, tag="ps")
                    for kk in range(KC):
                        nc.tensor.matmul(pq[:, :], wt[:, kk * 128:(kk + 1) * 128],
                                         hT[:, kk * CH:(kk + 1) * CH],
                                         start=(kk == 0), stop=(kk == KC - 1))
                    nc.vector.tensor_scalar_add(qT[:, m * CH:(m + 1) * CH], pq[:, :],
                                                qkb[:, m:m + 1])

            def emit_ag(tag, src, rows_out):
                if SKIP_AG:
                    out = dr.tile([rows_out, src.shape[1]], src.dtype, tag=tag + "o")
                    nc.sync.dma_start(out[0:src.shape[0], :], src[:, :])
                    return out
                out = dr.tile([rows_out, src.shape[1]], src.dtype, tag=tag + "o")
                nc.gpsimd.collective_compute(
                    "AllGather", ALU.bypass, replica_groups=RG,
                    ins=[src.opt()], outs=[out.opt()])
                return out

            def emit_unstage_k(ag_out):
                for r in range(4):
                    for pos in range(2):
                        gb = r if pos == 0 else 7 - r
                        nc.sync.dma_start(
                            kT8[:, :].rearrange("p (m j) -> p m j", m=8)
                            [:, :, gb * 128:(gb + 1) * 128],
                            ag_out[r * 2304:r * 2304 + D, pos * 128:(pos + 1) * 128]
                            .rearrange("(m p) j -> p m j", p=128))

            def emit_unstage_v(ag_out):
                for r in range(4):
                    for pos in range(2):
                        gb = r if pos == 0 else 7 - r
                        nc.sync.dma_start(
                            vst8[:, gb * 1280:(gb + 1) * 1280],
                            ag_out[r * 2304 + 1024 + pos * 640: r * 2304 + 1024 + (pos + 1) * 640, :]
                            .rearrange("(p x) c -> p (x c)", p=128))

            def emit_attn(layer):
                if SKIP_ATTN:
                    for c in range(KC):
                        nc.vector.tensor_copy(oT[:, c * CH:(c + 1) * CH],
                                              hT[:, c * CH:(c + 1) * CH])
                    return
                for m in range(8):
                    po = pso.tile([65, 512], F32, tag="po")
                    for h2 in range(2):
                        h = 2 * m + h2
                        prow = h2 * 64
                        qs2 = qT[prow:prow + 64, m * CH:(m + 1) * CH]
                        # group 0: jb 0..3, both iq halves -> pe cols jb*256
                        pe0 = pse.tile([128, 1024], F32, tag="pe")
                        for jb in range(4):
                            nc.tensor.matmul(
                                pe0[:, jb * 256:(jb + 1) * 256],
                                kT8[prow:prow + 64, (m * 8 + jb) * 128:(m * 8 + jb + 1) * 128],
                                qs2, start=True, stop=True, skip_group_check=True)
                        aT0 = tp4.tile([128, 1024], BF16, tag="aT")
                        if EXP_DVE:
                            nc.vector.tensor_copy(aT0[:, :], pe0[:, :])
                        else:
                            nc.scalar.activation(aT0[:, :], pe0[:, :], AF.Exp, scale=SCALE)
                        eng0 = nc.gpsimd if h2 == 0 else nc.vector
                        eng0.tensor_tensor(aT0[:, :], aT0[:, :], km_b[:, 0:1024], ALU.mult)
                        # group 1: jb 4..6 iq1-only at cols u*128, diag at 384+iq*128
                        pe1 = pse.tile([128, 1024], F32, tag="pe")
                        for u in range(3):
                            jb = 4 + u
                            nc.tensor.matmul(
                                pe1[:, u * 128:(u + 1) * 128],
                                kT8[prow:prow + 64, (m * 8 + jb) * 128:(m * 8 + jb + 1) * 128],
                                qT[prow:prow + 64, m * CH + 128: (m + 1) * CH],
                                start=True, stop=True, skip_group_check=True)
                        for iq in range(2):
                            nc.tensor.matmul(
                                pe1[:, 384 + iq * 128: 384 + (iq + 1) * 128],
                                kst[prow:prow + 64, m * CH + iq * 128: m * CH + iq * 128 + 128],
                                qT[prow:prow + 64, m * CH + iq * 128: m * CH + iq * 128 + 128],
                                start=True, stop=True, skip_group_check=True)
                        aT1 = tp4.tile([128, 1024], BF16, tag="aT")
                        if EXP_DVE:
                            nc.vector.tensor_copy(aT1[:, 0:640], pe1[:, 0:640])
                        else:
                            nc.scalar.activation(aT1[:, 0:640], pe1[:, 0:640], AF.Exp, scale=SCALE)
                        eng1 = nc.vector if h2 == 0 else nc.gpsimd
                        eng1.tensor_tensor(aT1[:, 0:640], aT1[:, 0:640],
                                           km_b[:, 1024:1664], ALU.mult)
                        # av accumulation into po[:, h2*256 + iq*128]
                        for iq in range(2):
                            oc = h2 * 256 + iq * 128
                            units = []
                            njb = NJ0 if iq == 0 else 4
                            for jb in range(njb):
                                units.append((vst8[:, jb * 1280 + h * 65: jb * 1280 + h * 65 + 65],
                                              aT0[:, jb * 256 + iq * 128: jb * 256 + iq * 128 + 128]))
                            if iq == 1:
                                for u in range(3):
                                    jb = 4 + u
                                    units.append((vst8[:, jb * 1280 + h * 65: jb * 1280 + h * 65 + 65],
                                                  aT1[:, u * 128:(u + 1) * 128]))
                            units.append((vstg[:, iq * 1280 + h * 65: iq * 1280 + h * 65 + 65],
                                          aT1[:, 384 + iq * 128: 384 + (iq + 1) * 128]))
                            for ui, (lhs, rhs) in enumerate(units):
                                nc.tensor.matmul(po[:, oc:oc + 128], lhs, rhs,
                                                 start=(ui == 0), stop=(ui == len(units) - 1),
                                                 skip_group_check=True)
                    # denominators for the whole pair
                    nc.vector.tensor_scalar_add(po[64:65, :], po[64:65, :], 1e-30)
                    drow = tp.tile([1, 512], F32R, tag="drow")
                    nc.vector.reciprocal(drow[:, :], po[64:65, :])
                    pb = ps.tile([64, 512], F32, tag="ps")
                    nc.tensor.matmul(pb[:, :], onesB[0:1, 0:64], drow[:, :],
                                     start=True, stop=True)
                    rb = tp.tile([64, 512], F32, tag="rb")
                    nc.vector.tensor_copy(rb[:, :], pb[:, :])
                    for h2 in range(2):
                        nc.vector.tensor_tensor(
                            oT[h2 * 64:(h2 + 1) * 64, m * CH:(m + 1) * CH],
                            po[0:64, h2 * 256:(h2 + 1) * 256],
                            rb[0:64, h2 * 256:(h2 + 1) * 256], ALU.mult)

            def emit_res(pq, br, c):
                """x[:, c] = (x + pq + bias) * m, engines alternating by c."""
                bst = consts[br]
                xs = xT[:, c * CH:(c + 1) * CH]
                nc.vector.scalar_tensor_tensor(xs, pq[:, :], bst[:, c:c + 1], xs,
                                                ALU.add, ALU.add)
                nc.gpsimd.tensor_tensor(xs, xs, mbT[:, :], ALU.mult)

            def emit_outproj(layer):
                for mb in range(8):
                    wt = wload(wout_d, layer, mb, KC, wp, "wa")
                    pq = ps.tile([128, CH], F32, tag="ps")
                    for kk in range(KC):
                        nc.tensor.matmul(pq[:, :], wt[:, kk * 128:(kk + 1) * 128],
                                         oT[:, kk * CH:(kk + 1) * CH],
                                         start=(kk == 0), stop=(kk == KC - 1))
                    emit_res(pq, 0, mb)

            def emit_ffn(layer):
                if SKIP_FFN:
                    return
                b1t = consts["b1"]
                for mb in range(32):
                    wt = wload(w1_d, layer, mb, KC, wp, "wa")
                    pf = ps.tile([128, CH], F32, tag="ps")
                    for kk in range(KC):
                        nc.tensor.matmul(pf[:, :], wt[:, kk * 128:(kk + 1) * 128],
                                         hT[:, kk * CH:(kk + 1) * CH],
                                         start=(kk == 0), stop=(kk == KC - 1))
                    nc.scalar.activation(ffT[:, mb * CH:(mb + 1) * CH], pf[:, :],
                                         AF.Gelu, bias=b1t[:, mb:mb + 1])
                for mb in range(8):
                    wt = wload(w2_d, layer, mb, 32, wf2, "wf2")
                    pq = ps.tile([128, CH], F32, tag="ps")
                    for kk in range(32):
                        nc.tensor.matmul(pq[:, :], wt[:, kk * 128:(kk + 1) * 128],
                                         ffT[:, kk * CH:(kk + 1) * CH],
                                         start=(kk == 0), stop=(kk == 31))
                    emit_res(pq, 1, mb)

            # ---- main loop ----
            for rep in range(REPS):
                for layer in range(L):
                    load_layer_consts(layer)
                    emit_adaln(layer, 0)
                    ag_in = dr.tile([2304, CH], FP8, tag="agi")
                    emit_k(layer, ag_in)
                    emit_v(layer, ag_in, load_wv(layer))
                    ag_out = emit_ag("ag", ag_in, 4 * 2304)
                    emit_q(layer)
                    emit_unstage_k(ag_out)
                    emit_unstage_v(ag_out)
                    emit_attn(layer)
                    emit_outproj(layer)
                    emit_adaln(layer, 1)
                    emit_ffn(layer)

            for c in range(KC):
                nc.sync.dma_start(out_d[c * 128:(c + 1) * 128, :].bitcast(F32R),
                                  xT[:, c * CH:(c + 1) * CH])

    nc.finalize()
    return nc


def get_nc():
    if "nc" not in _CACHED:
        _CACHED["nc"] = _build_nc()
    return _CACHED["nc"]


def _rearr(v, nch):
    """(..., nch*128) -> (..., 128, nch)."""
    v = np.asarray(v, dtype=np.float32)
    return np.ascontiguousarray(v.reshape(*v.shape[:-1], nch, 128).swapaxes(-1, -2))


def _strips(w, nmb, nkc):
    """[L, K, M] -> [L, nmb, nkc, 128, 128] fp16 with [l,mb,kc,p,c]=w[l,kc*128+p,mb*128+c]."""
    Lw = w.shape[0]
    a = w.reshape(Lw, nkc, 128, nmb, 128).transpose(0, 3, 2, 1, 4)
    return np.ascontiguousarray(a.astype(np.float16))


def make_in_maps(x, m, l, Wqkv, Wout, bout, adaln_attn, adaln_ffn, W1, b1, W2, b2):
    x = np.asarray(x, np.float32)
    m = np.asarray(m, np.float32)
    l = np.asarray(l)
    Wqkv = np.asarray(Wqkv, np.float32)
    Wout = np.asarray(Wout, np.float32)
    bout = np.asarray(bout, np.float32)
    adaln_attn = np.asarray(adaln_attn, np.float32)
    adaln_ffn = np.asarray(adaln_ffn, np.float32)
    W1 = np.asarray(W1, np.float32)
    b1 = np.asarray(b1, np.float32)
    W2 = np.asarray(W2, np.float32)
    b2 = np.asarray(b2, np.float32)

    causal01 = (np.arange(128)[:, None] <= np.arange(128)[None, :]).astype(np.float32)
    onescol = np.ones((128, 1), np.float32)
    kbias = np.full((128, 1), -1.0 / (2.0 * ADALN_K ** 0.5), np.float32)

    per_batch = {}
    for b in range(2):
        lv = int(l[b])
        ga = adaln_attn[:, lv, :]
        gf = adaln_ffn[:, lv, :]
        g1a = (2.0 * np.exp(ga[:, :D])).astype(np.float32)
        g1f = (2.0 * np.exp(gf[:, :D])).astype(np.float32)
        # mod = (sqrt(K)t - 1/(2 sqrt(K)))^2 = -(t - K t^2) + 1/(4K): sign into
        # gamma, constant into beta.
        bea = (ga[:, D:] + g1a / (4.0 * ADALN_K)).astype(np.float32)
        bef = (gf[:, D:] + g1f / (4.0 * ADALN_K)).astype(np.float32)
        g1a, g1f = -g1a, -g1f
        wqkv_s = Wqkv * g1a[:, :, None]
        w1_s = W1 * g1f[:, :, None]
        wv_full = Wqkv[:, :, 2 * D:3 * D]
        vc = np.einsum("ldf,ld->lf", wv_full, bea)
        bout_c = bout + np.einsum("ldf,ld->lf", Wout, vc)
        qkbias = np.einsum("ldf,ld->lf", Wqkv[:, :, :2 * D], bea).astype(np.float32)
        # k staged as KSC*(k+bias) for the fp8 AllGather; ACT applies
        # scale=KSC to the psum, so the k biases must be pre-scaled too
        qkbias = qkbias.copy()
        qkbias[:, D:] *= KSC
        b1_c = (b1 + np.einsum("ldf,ld->lf", W1, bef)).astype(np.float32)

        wqk_r = _strips(wqkv_s[:, :, :2 * D], 16, KC)
        wv_r = np.ascontiguousarray(
            wqkv_s[:, :, 2 * D:].reshape(L, KC, 128, D).transpose(0, 2, 1, 3)
            .astype(np.float16))
        wout_r = _strips(Wout, 8, KC)
        w1_r = _strips(w1_s, 32, KC)
        w2_r = _strips(W2, 8, 32)
        bsum_t = _rearr(np.stack([bout_c, b2], axis=1), 8)
        per_batch[b] = dict(
            wqk=wqk_r, wv=wv_r, wout=wout_r, w1=w1_r, w2=w2_r,
            qkb=_rearr(qkbias, 16), b1t=_rearr(b1_c, 32), bsum=bsum_t)

    in_maps = []
    for core in range(8):
        b, s = core // 4, core % 4
        blocks = [s, 7 - s]
        pb = per_batch[b]
        cols = np.concatenate([np.arange(bk * 128, (bk + 1) * 128) for bk in blocks])
        xTc = np.ascontiguousarray(x[b].T[:, cols])
        mrow = np.ascontiguousarray(m[b, cols, 0].reshape(1, CH))
        # KSC folds into v rows and the denominator ones-column (ratio exact)
        mqp = KSC * np.stack([m[b, bk * 128:(bk + 1) * 128, 0] for bk in blocks],
                             axis=1).astype(np.float32)
        kmask = np.zeros((128, 1664), np.float32)
        for jb in range(4):                      # group 0: both iq halves
            for iq, qb in enumerate(blocks):
                if jb < qb:
                    kmask[:, jb * 256 + iq * 128: jb * 256 + (iq + 1) * 128] = 1.0
        for u in range(3):                       # group 1: iq1-only jb 4..6
            if 4 + u < blocks[1]:
                kmask[:, 1024 + u * 128: 1024 + (u + 1) * 128] = 1.0
        for iq in range(2):                      # group 1: diag causal triangles
            kmask[:, 1408 + iq * 128: 1408 + (iq + 1) * 128] = causal01
        import ml_dtypes
        in_maps.append({
            "xT": xTc, "wqk": pb["wqk"], "wv": pb["wv"], "wout": pb["wout"],
            "w1": pb["w1"], "w2": pb["w2"], "qkb": pb["qkb"], "b1t": pb["b1t"],
            "bsum": pb["bsum"], "kmask": kmask.astype(ml_dtypes.bfloat16),
            "mqp": mqp, "mrow": mrow,
            "onescol": onescol, "kbias": kbias,
        })
    return in_maps


def kernel(**inputs):
    nc = get_nc()
    in_maps = make_in_maps(**inputs)
    res = run_bass_kernel_spmd(nc, in_maps, core_ids=list(range(8)))
    out = np.zeros((2, T, D), np.float32)
    for core in range(8):
        b, s = core // 4, core % 4
        o = res.results[core]["out_xT"]          # [D, CH]
        for iq, bk in enumerate([s, 7 - s]):
            out[b, bk * 128:(bk + 1) * 128, :] = o[:, iq * 128:(iq + 1) * 128].T
    return np.ascontiguousarray(out)



# revision 35
# speedup vs baseline: 1.0104x; 1.0104x over previous
"""Trainium2 Bass kernel for the 4-layer adaLN causal transformer (v3.2).

v3.2 = v3 + fp8e4m3 AllGather payload, consumed directly by the PE:
k and v are staged to the collective in fp8 (1 byte), halving the gathered
bytes (4.7MB -> 2.36MB out per layer), and the gathered fp8 tiles feed the
attention matmuls directly as lhsT (mixed fp8 x f16/bf16 matmuls -- no
conversion pass; results are bit-identical to explicit DVE conversion).
k is scaled by KSC=8 into fp8 range (bias pre-scaled on host, exp scale
divided by KSC); v and the denominator ones-column are both scaled by KSC so
the softmax ratio is exact. KSC must keep |KSC*(k+bias)| well under 448 --
e4m3 saturates to NaN (KSC=16 overflowed; max |k+bias| ~ 22). Measured
rel err 1.35e-2 (budget 2e-2); paired A/B slope ~ -13% vs v3.

v3: attention restructured for instruction efficiency: e-matmuls write wide
PSUM groups ([128,1024]/[128,640] per head), ONE exp per group, bf16
kill-mask multiplies (causal block-kill + diag triangle) on DVE/Pool, key
padding folded into zeroed v rows/ones at staging, softmax denominators
processed per head-pair.

Sharding: sequence-parallel. 8 cores = 2 batch groups x 4 token shards.
Core (b, s) owns query blocks {s, 7-s} of batch b (zigzag for causal load
balance; every core sees exactly 9 key-blocks of true attention work).
Weights are replicated (full, adaLN-folded per batch, fp16); there are NO
AllReduces. Per layer the only collectives are two small AllGathers (k and
v across the 4 shards), and the diagonal attention blocks use local k/v so
they start before the AllGather lands.

Activations stay transposed xT[d, t_local] (f32r residual stream, fp16
modulated hT). The adaLN affine is folded into Wqkv/W1 on the host exactly
as in the TP kernel: gamma scales weight rows, beta enters as per-partition
biases on q/k and the gelu, and the v-beta contribution folds into bout.
Attention avoids max-subtraction: exp() rides the ACT bias with -1e30 key
masks; aT/v are bf16 (range) while everything else is fp16.

Residuals are masked every sub-block ((x+f)*m, matching the reference), so
padded-token columns stay exactly 0 and nothing can overflow -> no clamps.

Uniform SPMD program: per-core differences live only in input data
(madd2/mdiag mask tables, xT slices); off-diagonal passes that a core does
not need are killed by -1e30 masks (~25% wasted attention work).
"""

import os
import numpy as np

import concourse.bacc as bacc
import concourse.mybir as mybir
from concourse.tile import TileContext
from concourse.bass_utils import run_bass_kernel_spmd

F32 = mybir.dt.float32
F32R = mybir.dt.float32r
F16 = mybir.dt.float16
BF16 = mybir.dt.bfloat16
FP8 = mybir.dt.float8e4
AF = mybir.ActivationFunctionType
ALU = mybir.AluOpType

D = 1024
T = 1024
L = 4
CH = 256            # local tokens per core (2 blocks of 128)
KC = 8              # d_model chunks
ADALN_K = 0.1
EPS = 1e-5
KSC = 8.0           # k/v pre-scale into fp8e4m3 range for the AllGather
                    # (must keep KSC*(k+bias) well under 448: e4m3 saturates to NaN)
SCALE = 0.125 / KSC
RG = [[0, 1, 2, 3], [4, 5, 6, 7]]
NJ0 = 3             # uniform off-diag key blocks for iq=0 (covers qb=s<=3)
NJ1 = 7             # for iq=1 (covers qb=7-s<=7)
REPS = int(os.environ.get("BK2_REPS", "1"))
SKIP_AG = bool(int(os.environ.get("BK2_SKIP_AG", "0")))    # timing ablation
SKIP_ATTN = bool(int(os.environ.get("BK2_SKIP_ATTN", "0")))
SKIP_FFN = bool(int(os.environ.get("BK2_SKIP_FFN", "0")))
WONCE = bool(int(os.environ.get("BK2_WONCE", "0")))        # timing ablation: 1 DMA per weight kind/layer
EXP_DVE = bool(int(os.environ.get("BK2_EXP_DVE", "0")))    # timing ablation: exp -> DVE copy

_CACHED = {}


def _build_nc():
    nc = bacc.Bacc(target_bir_lowering=False, debug=False)

    xT_d = nc.dram_tensor("xT", [D, CH], F32, kind="ExternalInput")
    # weights pre-rearranged on host: [L, mb, kc, p, c] so strips DMA contiguously
    wqk_d = nc.dram_tensor("wqk", [L, 16, 128, KC, 128], F16, kind="ExternalInput")
    wv_d = nc.dram_tensor("wv", [L, 128, KC, D], F16, kind="ExternalInput")
    wout_d = nc.dram_tensor("wout", [L, 8, 128, KC, 128], F16, kind="ExternalInput")
    w1_d = nc.dram_tensor("w1", [L, 32, 128, KC, 128], F16, kind="ExternalInput")
    w2_d = nc.dram_tensor("w2", [L, 8, 128, 32, 128], F16, kind="ExternalInput")
    qkb_d = nc.dram_tensor("qkb", [L, 128, 16], F32, kind="ExternalInput")
    b1t_d = nc.dram_tensor("b1t", [L, 128, 32], F32, kind="ExternalInput")
    bsum_d = nc.dram_tensor("bsum", [L, 2, 128, 8], F32, kind="ExternalInput")
    kmask_d = nc.dram_tensor("kmask", [128, 1664], BF16, kind="ExternalInput")
    mqp_d = nc.dram_tensor("mqp", [128, 2], F32, kind="ExternalInput")
    mrow_d = nc.dram_tensor("mrow", [1, CH], F32, kind="ExternalInput")
    ones_d = nc.dram_tensor("onescol", [128, 1], F32, kind="ExternalInput")
    kb_d = nc.dram_tensor("kbias", [128, 1], F32, kind="ExternalInput")
    out_d = nc.dram_tensor("out_xT", [D, CH], F32, kind="ExternalOutput")

    with TileContext(nc) as tc:
        with nc.allow_low_precision("fp16/bf16 intermediates by design"), \
             tc.tile_pool(name="pers", bufs=1) as pers, \
             tc.tile_pool(name="wp", bufs=8) as wp, \
             tc.tile_pool(name="wf2", bufs=3) as wf2, \
             tc.tile_pool(name="cst", bufs=8) as cst, \
             tc.tile_pool(name="tp", bufs=3) as tp, \
             tc.tile_pool(name="tp4", bufs=6) as tp4, \
             tc.tile_pool(name="wvp", bufs=1) as wvp, \
             tc.tile_pool(name="ps", bufs=2, space="PSUM") as ps, \
             tc.tile_pool(name="pse", bufs=2, space="PSUM") as pse, \
             tc.tile_pool(name="pso", bufs=2, space="PSUM") as pso, \
             tc.tile_pool(name="dr", bufs=2, space="DRAM") as dr:

            # ---- persistent tiles ----
            xT = pers.tile([128, KC * CH], F32R, tag="xT")
            hT = pers.tile([128, KC * CH], F16, tag="hT")
            qT = pers.tile([128, 8 * CH], F16, tag="qT")       # pair m at m*CH, iq block at +iq*128
            kst = pers.tile([128, 8 * CH], F16, tag="kst")     # local k*KSC (also diag lhsT)
            kst8 = pers.tile([128, 8 * CH], FP8, tag="kst8")   # fp8 AG payload (k)
            kT8 = pers.tile([128, 64 * 128], FP8, tag="kT8")   # gathered k (fp8), (m,jb) at (m*8+jb)*128
            vstg = pers.tile([128, 2 * 1280], BF16, tag="vstg")  # local v*KSC+ones (65/head, pad 1280)
            vstg8 = pers.tile([128, 2 * 1280], FP8, tag="vstg8")  # fp8 AG payload (v)
            vst8 = pers.tile([128, 8 * 1280], FP8, tag="vst8")    # gathered v (fp8): (jb, h) at jb*1280+h*65
            oT = pers.tile([128, KC * CH], F16, tag="oT")
            ffT = pers.tile([128, 32 * CH], F16, tag="ffT")
            onesK = pers.tile([128, 1], F32R, tag="onesK")
            onesB = pers.tile([1, 128], F32R, tag="onesB")
            km_b = pers.tile([128, 1664], BF16, tag="kmb")
            kb_t = pers.tile([128, 1], F32, tag="kb")
            mqp_t = pers.tile([128, 2], F32, tag="mqp")
            mrow_r = pers.tile([1, CH], F32R, tag="mrow")
            mbT = pers.tile([128, CH], F32, tag="mbT")         # mask broadcast

            nc.sync.dma_start(onesK[:, :], ones_d[:, :].bitcast(F32R))
            nc.sync.dma_start(onesB[:, :], ones_d[:, 0:1].bitcast(F32R).rearrange("p 1 -> 1 p"))
            nc.sync.dma_start(km_b[:, :], kmask_d[:, :])
            nc.sync.dma_start(kb_t[:, :], kb_d[:, :])
            nc.sync.dma_start(mqp_t[:, :], mqp_d[:, :])
            nc.sync.dma_start(mrow_r[:, :], mrow_d[:, :].bitcast(F32R))
            # ones columns for the softmax denominators (x=64 of each 65-strip)
            for blk in range(2):
                ones_ap = (vstg[:, blk * 1280: blk * 1280 + 1040]
                           .rearrange("p (s x) -> p s x", x=65)[:, :, 64:65])
                nc.vector.memset(ones_ap, 1.0)
                nc.vector.tensor_scalar_mul(ones_ap, ones_ap, mqp_t[:, blk:blk + 1])
                nc.vector.memset(vstg[:, blk * 1280 + 1040:(blk + 1) * 1280], 0.0)
            for c in range(KC):
                nc.sync.dma_start(
                    xT[:, c * CH:(c + 1) * CH],
                    xT_d[c * 128:(c + 1) * 128, :].bitcast(F32R))
            # mask broadcast [128, CH]
            pm = ps.tile([128, CH], F32, tag="ps")
            nc.tensor.matmul(pm[:, :], onesB[:, :], mrow_r[:, :], start=True, stop=True)
            nc.vector.tensor_copy(mbT[:, :], pm[:, :])

            consts = {}

            def load_layer_consts(layer):
                qkb = cst.tile([128, 16], F32, tag="qkb")
                nc.sync.dma_start(qkb[:, :], qkb_d[layer])
                b1t = cst.tile([128, 32], F32, tag="b1")
                nc.sync.dma_start(b1t[:, :], b1t_d[layer])
                bs0 = cst.tile([128, 8], F32, tag="bs0")
                nc.sync.dma_start(bs0[:, :], bsum_d[layer, 0])
                bs1 = cst.tile([128, 8], F32, tag="bs1")
                nc.sync.dma_start(bs1[:, :], bsum_d[layer, 1])
                consts.update({"qkb": qkb, "b1": b1t, 0: bs0, 1: bs1})

            wcache = {}

            def wload(w_d, layer, mb, kcnt, pool, tag):
                key = id(w_d)
                if WONCE and key in wcache:
                    return wcache[key]
                wt = pool.tile([128, kcnt * 128], F16, tag=tag)
                nc.sync.dma_start(
                    wt[:, :].rearrange("p (k c) -> p k c", k=kcnt), w_d[layer, mb])
                wcache[key] = wt
                return wt

            def emit_adaln(layer, sb):
                """hT = mod(norm(xT)); affine folded into consuming weights."""
                ps_sum = ps.tile([1, CH], F32, tag="ps")
                ps_sq = ps.tile([1, CH], F32, tag="ps")
                for c in range(KC):
                    xs = xT[:, c * CH:(c + 1) * CH]
                    xsq = tp.tile([128, CH], F32R, tag="xsq")
                    nc.scalar.activation(xsq[:, :], xs, AF.Square)
                    nc.tensor.matmul(ps_sum[:, :], onesK[:, :], xs,
                                     start=(c == 0), stop=(c == KC - 1))
                    nc.tensor.matmul(ps_sq[:, :], onesK[:, :], xsq[:, :],
                                     start=(c == 0), stop=(c == KC - 1))
                murow = tp.tile([1, CH], F32R, tag="murow")
                nc.scalar.mul(murow[:, :], ps_sum[0:1, :], 1.0 / D)
                m2row = tp.tile([1, CH], F32, tag="m2row")
                nc.scalar.mul(m2row[:, :], ps_sq[0:1, :], 1.0 / D)
                musq = tp.tile([1, CH], F32, tag="musq")
                nc.vector.tensor_tensor(musq[:, :], murow[:, :], murow[:, :], ALU.mult)
                nc.vector.tensor_tensor(m2row[:, :], m2row[:, :], musq[:, :], ALU.subtract)
                nc.vector.tensor_scalar_add(m2row[:, :], m2row[:, :], EPS)
                nc.scalar.activation(musq[:, :], m2row[:, :], AF.Sqrt)
                rrow = tp.tile([1, CH], F32R, tag="rrow")
                nc.vector.reciprocal(rrow[:, :], musq[:, :])
                mrs = tp.tile([1, CH], F32R, tag="mrs")
                nc.vector.tensor_tensor(mrs[:, :], murow[:, :], rrow[:, :], ALU.mult)
                ps_rs = ps.tile([128, CH], F32, tag="ps")
                nc.tensor.matmul(ps_rs[:, :], onesB[:, :], rrow[:, :], start=True, stop=True)
                ps_mrs = ps.tile([128, CH], F32, tag="ps")
                nc.tensor.matmul(ps_mrs[:, :], onesB[:, :], mrs[:, :], start=True, stop=True)
                rsb = tp.tile([128, CH], F32, tag="rsb")
                nc.vector.tensor_copy(rsb[:, :], ps_rs[:, :])
                mrsb = tp.tile([128, CH], F32, tag="mrsb")
                nc.vector.tensor_copy(mrsb[:, :], ps_mrs[:, :])
                for c in range(KC):
                    xs = xT[:, c * CH:(c + 1) * CH]
                    t0 = tp.tile([128, CH], F32, tag="t0")
                    eng = nc.vector if c % 2 == 0 else nc.gpsimd
                    eng.tensor_tensor(t0[:, :], xs, rsb[:, :], ALU.mult)
                    eng.tensor_tensor(t0[:, :], t0[:, :], mrsb[:, :], ALU.subtract)
                    nc.scalar.activation(
                        hT[:, c * CH:(c + 1) * CH], t0[:, :],
                        AF.Square, scale=float(ADALN_K ** 0.5), bias=kb_t[:, 0:1])

            def emit_k(layer, ag_in):
                qkb = consts["qkb"]
                for m in range(8):
                    mb = 8 + m
                    wt = wload(wqk_d, layer, mb, KC, wp, "wa")
                    pq = ps.tile([128, CH], F32, tag="ps")
                    for kk in range(KC):
                        nc.tensor.matmul(pq[:, :], wt[:, kk * 128:(kk + 1) * 128],
                                         hT[:, kk * CH:(kk + 1) * CH],
                                         start=(kk == 0), stop=(kk == KC - 1))
                    # kst = KSC*(k + bias): scale into fp8 range (bias pre-scaled on host)
                    nc.scalar.activation(kst[:, m * CH:(m + 1) * CH], pq[:, :],
                                         AF.Identity, scale=KSC, bias=qkb[:, mb:mb + 1])
                    eng = nc.vector if m % 2 == 0 else nc.gpsimd
                    eng.tensor_copy(kst8[:, m * CH:(m + 1) * CH],
                                    kst[:, m * CH:(m + 1) * CH])
                    nc.sync.dma_start(ag_in[m * 128:(m + 1) * 128, :],
                                      kst8[:, m * CH:(m + 1) * CH])

            def load_wv(layer):
                wvL = wvp.tile([128, KC * D], F16, tag="wv")
                nc.sync.dma_start(
                    wvL[:, :].rearrange("p (k c) -> p k c", k=KC), wv_d[layer])
                return wvL

            def emit_v(layer, ag_in, wvL):
                for blk in range(2):
                    for half in range(2):
                        pv = ps.tile([128, 512], F32, tag="ps")
                        for kc in range(KC):
                            nc.tensor.matmul(
                                pv[:, :],
                                hT[:, kc * CH + blk * 128: kc * CH + blk * 128 + 128],
                                wvL[:, kc * D + half * 512: kc * D + (half + 1) * 512],
                                start=(kc == 0), stop=(kc == KC - 1))
                        # v psum [128tok, 512 vd] -> vstg strips, zeroing pad-token rows
                        nc.vector.tensor_scalar_mul(
                            vstg[:, blk * 1280 + half * 8 * 65: blk * 1280 + (half * 8 + 8) * 65]
                            .rearrange("p (h x) -> p h x", x=65)[:, :, 0:64],
                            pv[:, :].rearrange("p (h d) -> p h d", d=64),
                            mqp_t[:, blk:blk + 1])
                    eng = nc.vector if blk == 0 else nc.gpsimd
                    eng.tensor_copy(vstg8[:, blk * 1280:(blk + 1) * 1280],
                                    vstg[:, blk * 1280:(blk + 1) * 1280])
                    nc.sync.dma_start(
                        ag_in[1024 + blk * 640: 1024 + (blk + 1) * 640, :]
                        .rearrange("(p x) c -> p (x c)", p=128),
                        vstg8[:, blk * 1280:(blk + 1) * 1280])

            def emit_q(layer):
                qkb = consts["qkb"]
                for m in range(8):
                    wt = wload(wqk_d, layer, m, KC, wp, "wa")
                    pq = ps.tile([128, CH], F32, tag="ps")
                    for kk in range(KC):
                        nc.tensor.matmul(pq[:, :], wt[:, kk * 128:(kk + 1) * 128],
                                         hT[:, kk * CH:(kk + 1) * CH],
                                         start=(kk == 0), stop=(kk == KC - 1))
                    nc.vector.tensor_scalar_add(qT[:, m * CH:(m + 1) * CH], pq[:, :],
                                                qkb[:, m:m + 1])

            def emit_ag(tag, src, rows_out):
                if SKIP_AG:
                    out = dr.tile([rows_out, src.shape[1]], src.dtype, tag=tag + "o")
                    nc.sync.dma_start(out[0:src.shape[0], :], src[:, :])
                    return out
                out = dr.tile([rows_out, src.shape[1]], src.dtype, tag=tag + "o")
                nc.gpsimd.collective_compute(
                    "AllGather", ALU.bypass, replica_groups=RG,
                    ins=[src.opt()], outs=[out.opt()])
                return out

            def emit_unstage_k(ag_out):
                for r in range(4):
                    for pos in range(2):
                        gb = r if pos == 0 else 7 - r
                        nc.sync.dma_start(
                            kT8[:, :].rearrange("p (m j) -> p m j", m=8)
                            [:, :, gb * 128:(gb + 1) * 128],
                            ag_out[r * 2304:r * 2304 + D, pos * 128:(pos + 1) * 128]
                            .rearrange("(m p) j -> p m j", p=128))

            def emit_unstage_v(ag_out):
                for r in range(4):
                    for pos in range(2):
                        gb = r if pos == 0 else 7 - r
                        nc.sync.dma_start(
                            vst8[:, gb * 1280:(gb + 1) * 1280],
                            ag_out[r * 2304 + 1024 + pos * 640: r * 2304 + 1024 + (pos + 1) * 640, :]
                            .rearrange("(p x) c -> p (x c)", p=128))

            def emit_attn(layer):
                if SKIP_ATTN:
                    for c in range(KC):
                        nc.vector.tensor_copy(oT[:, c * CH:(c + 1) * CH],
                                              hT[:, c * CH:(c + 1) * CH])
                    return
                for m in range(8):
                    po = pso.tile([65, 512], F32, tag="po")
                    for h2 in range(2):
                        h = 2 * m + h2
                        prow = h2 * 64
                        qs2 = qT[prow:prow + 64, m * CH:(m + 1) * CH]
                        # group 0: jb 0..3, both iq halves -> pe cols jb*256
                        pe0 = pse.tile([128, 1024], F32, tag="pe")
                        for jb in range(4):
                            nc.tensor.matmul(
                                pe0[:, jb * 256:(jb + 1) * 256],
                                kT8[prow:prow + 64, (m * 8 + jb) * 128:(m * 8 + jb + 1) * 128],
                                qs2, start=True, stop=True, skip_group_check=True)
                        aT0 = tp4.tile([128, 1024], BF16, tag="aT")
                        if EXP_DVE:
                            nc.vector.tensor_copy(aT0[:, :], pe0[:, :])
                        else:
                            nc.scalar.activation(aT0[:, :], pe0[:, :], AF.Exp, scale=SCALE)
                        eng0 = nc.gpsimd if h2 == 0 else nc.vector
                        eng0.tensor_tensor(aT0[:, :], aT0[:, :], km_b[:, 0:1024], ALU.mult)
                        # group 1: jb 4..6 iq1-only at cols u*128, diag at 384+iq*128
                        pe1 = pse.tile([128, 1024], F32, tag="pe")
                        for u in range(3):
                            jb = 4 + u
                            nc.tensor.matmul(
                                pe1[:, u * 128:(u + 1) * 128],
                                kT8[prow:prow + 64, (m * 8 + jb) * 128:(m * 8 + jb + 1) * 128],
                                qT[prow:prow + 64, m * CH + 128: (m + 1) * CH],
                                start=True, stop=True, skip_group_check=True)
                        for iq in range(2):
                            nc.tensor.matmul(
                                pe1[:, 384 + iq * 128: 384 + (iq + 1) * 128],
                                kst[prow:prow + 64, m * CH + iq * 128: m * CH + iq * 128 + 128],
                                qT[prow:prow + 64, m * CH + iq * 128: m * CH + iq * 128 + 128],
                                start=True, stop=True, skip_group_check=True)
                        aT1 = tp4.tile([128, 1024], BF16, tag="aT")
                        if EXP_DVE:
                            nc.vector.tensor_copy(aT1[:, 0:640], pe1[:, 0:640])
                        else:
                            nc.scalar.activation(aT1[:, 0:640], pe1[:, 0:640], AF.Exp, scale=SCALE)
                        eng1 = nc.vector if h2 == 0 else nc.gpsimd
                        eng1.tensor_tensor(aT1[:, 0:640], aT1[:, 0:640],
                                           km_b[:, 1024:1664], ALU.mult)
                        # av accumulation into po[:, h2*256 + iq*128]
                        for iq in range(2):
                            oc = h2 * 256 + iq * 128
                            units = []
                            njb = NJ0 if iq == 0 else 4
                            for jb in range(njb):
                                units.append((vst8[:, jb * 1280 + h * 65: jb * 1280 + h * 65 + 65],
                                              aT0[:, jb * 256 + iq * 128: jb * 256 + iq * 128 + 128]))
                            if iq == 1:
                                for u in range(3):
                                    jb = 4 + u
                                    units.append((vst8[:, jb * 1280 + h * 65: jb * 1280 + h * 65 + 65],
                                                  aT1[:, u * 128:(u + 1) * 128]))
                            units.append((vstg[:, iq * 1280 + h * 65: iq * 1280 + h * 65 + 65],
                                          aT1[:, 384 + iq * 128: 384 + (iq + 1) * 128]))
                            for ui, (lhs, rhs) in enumerate(units):
                                nc.tensor.matmul(po[:, oc:oc + 128], lhs, rhs,
                                                 start=(ui == 0), stop=(ui == len(units) - 1),
                                                 skip_group_check=True)
                    # denominators for the whole pair
                    nc.vector.tensor_scalar_add(po[64:65, :], po[64:65, :], 1e-30)
                    drow = tp.tile([1, 512], F32R, tag="drow")
                    nc.vector.reciprocal(drow[:, :], po[64:65, :])
                    pb = ps.tile([64, 512], F32, tag="ps")
                    nc.tensor.matmul(pb[:, :], onesB[0:1, 0:64], drow[:, :],
                                     start=True, stop=True)
                    rb = tp.tile([64, 512], F32, tag="rb")
                    nc.vector.tensor_copy(rb[:, :], pb[:, :])
                    for h2 in range(2):
                        nc.vector.tensor_tensor(
                            oT[h2 * 64:(h2 + 1) * 64, m * CH:(m + 1) * CH],
                            po[0:64, h2 * 256:(h2 + 1) * 256],
                            rb[0:64, h2 * 256:(h2 + 1) * 256], ALU.mult)

            def emit_res(pq, br, c):
                """x[:, c] = (x + pq + bias) * m, engines alternating by c."""
                bst = consts[br]
                xs = xT[:, c * CH:(c + 1) * CH]
                nc.vector.scalar_tensor_tensor(xs, pq[:, :], bst[:, c:c + 1], xs,
                                                ALU.add, ALU.add)
                nc.gpsimd.tensor_tensor(xs, xs, mbT[:, :], ALU.mult)

            def emit_outproj(layer):
                for mb in range(8):
                    wt = wload(wout_d, layer, mb, KC, wp, "wa")
                    pq = ps.tile([128, CH], F32, tag="ps")
                    for kk in range(KC):
                        nc.tensor.matmul(pq[:, :], wt[:, kk * 128:(kk + 1) * 128],
                                         oT[:, kk * CH:(kk + 1) * CH],
                                         start=(kk == 0), stop=(kk == KC - 1))
                    emit_res(pq, 0, mb)

            def emit_ffn(layer):
                if SKIP_FFN:
                    return
                b1t = consts["b1"]
                for mb in range(32):
                    wt = wload(w1_d, layer, mb, KC, wp, "wa")
                    pf = ps.tile([128, CH], F32, tag="ps")
                    for kk in range(KC):
                        nc.tensor.matmul(pf[:, :], wt[:, kk * 128:(kk + 1) * 128],
                                         hT[:, kk * CH:(kk + 1) * CH],
                                         start=(kk == 0), stop=(kk == KC - 1))
                    nc.scalar.activation(ffT[:, mb * CH:(mb + 1) * CH], pf[:, :],
                                         AF.Gelu, bias=b1t[:, mb:mb + 1])
                for mb in range(8):
                    wt = wload(w2_d, layer, mb, 32, wf2, "wf2")
                    pq = ps.tile([128, CH], F32, tag="ps")
                    for kk in range(32):
                        nc.tensor.matmul(pq[:, :], wt[:, kk * 128:(kk + 1) * 128],
                                         ffT[:, kk * CH:(kk + 1) * CH],
                                         start=(kk == 0), stop=(kk == 31))
                    emit_res(pq, 1, mb)

            # ---- main loop ----
            for rep in range(REPS):
                for layer in range(L):
                    load_layer_consts(layer)
                    emit_adaln(layer, 0)
                    ag_in = dr.tile([2304, CH], FP8, tag="agi")
                    emit_k(layer, ag_in)
                    emit_v(layer, ag_in, load_wv(layer))
                    ag_out = emit_ag("ag", ag_in, 4 * 2304)
                    emit_q(layer)
                    emit_unstage_k(ag_out)
                    emit_unstage_v(ag_out)
                    emit_attn(layer)
                    emit_outproj(layer)
                    emit_adaln(layer, 1)
                    emit_ffn(layer)

            for c in range(KC):
                nc.sync.dma_start(out_d[c * 128:(c + 1) * 128, :].bitcast(F32R),
                                  xT[:, c * CH:(c + 1) * CH])

    nc.finalize()
    return nc


def get_nc():
    if "nc" not in _CACHED:
        _CACHED["nc"] = _build_nc()
    return _CACHED["nc"]


def _rearr(v, nch):
    """(..., nch*128) -> (..., 128, nch)."""
    v = np.asarray(v, dtype=np.float32)
    return np.ascontiguousarray(v.reshape(*v.shape[:-1], nch, 128).swapaxes(-1, -2))


def _strips(w, nmb, nkc):
    """[L, K, M] -> [L, nmb, nkc, 128, 128] fp16 with [l,mb,kc,p,c]=w[l,kc*128+p,mb*128+c]."""
    Lw = w.shape[0]
    a = w.reshape(Lw, nkc, 128, nmb, 128).transpose(0, 3, 2, 1, 4)
    return np.ascontiguousarray(a.astype(np.float16))


def make_in_maps(x, m, l, Wqkv, Wout, bout, adaln_attn, adaln_ffn, W1, b1, W2, b2):
    x = np.asarray(x, np.float32)
    m = np.asarray(m, np.float32)
    l = np.asarray(l)
    Wqkv = np.asarray(Wqkv, np.float32)
    Wout = np.asarray(Wout, np.float32)
    bout = np.asarray(bout, np.float32)
    adaln_attn = np.asarray(adaln_attn, np.float32)
    adaln_ffn = np.asarray(adaln_ffn, np.float32)
    W1 = np.asarray(W1, np.float32)
    b1 = np.asarray(b1, np.float32)
    W2 = np.asarray(W2, np.float32)
    b2 = np.asarray(b2, np.float32)

    causal01 = (np.arange(128)[:, None] <= np.arange(128)[None, :]).astype(np.float32)
    onescol = np.ones((128, 1), np.float32)
    kbias = np.full((128, 1), -1.0 / (2.0 * ADALN_K ** 0.5), np.float32)

    per_batch = {}
    for b in range(2):
        lv = int(l[b])
        ga = adaln_attn[:, lv, :]
        gf = adaln_ffn[:, lv, :]
        g1a = (2.0 * np.exp(ga[:, :D])).astype(np.float32)
        g1f = (2.0 * np.exp(gf[:, :D])).astype(np.float32)
        # mod = (sqrt(K)t - 1/(2 sqrt(K)))^2 = -(t - K t^2) + 1/(4K): sign into
        # gamma, constant into beta.
        bea = (ga[:, D:] + g1a / (4.0 * ADALN_K)).astype(np.float32)
        bef = (gf[:, D:] + g1f / (4.0 * ADALN_K)).astype(np.float32)
        g1a, g1f = -g1a, -g1f
        wqkv_s = Wqkv * g1a[:, :, None]
        w1_s = W1 * g1f[:, :, None]
        wv_full = Wqkv[:, :, 2 * D:3 * D]
        vc = np.einsum("ldf,ld->lf", wv_full, bea)
        bout_c = bout + np.einsum("ldf,ld->lf", Wout, vc)
        qkbias = np.einsum("ldf,ld->lf", Wqkv[:, :, :2 * D], bea).astype(np.float32)
        # k staged as KSC*(k+bias) for the fp8 AllGather; ACT applies
        # scale=KSC to the psum, so the k biases must be pre-scaled too
        qkbias = qkbias.copy()
        qkbias[:, D:] *= KSC
        b1_c = (b1 + np.einsum("ldf,ld->lf", W1, bef)).astype(np.float32)

        wqk_r = _strips(wqkv_s[:, :, :2 * D], 16, KC)
        wv_r = np.ascontiguousarray(
            wqkv_s[:, :, 2 * D:].reshape(L, KC, 128, D).transpose(0, 2, 1, 3)
            .astype(np.float16))
        wout_r = _strips(Wout, 8, KC)
        w1_r = _strips(w1_s, 32, KC)
        w2_r = _strips(W2, 8, 32)
        bsum_t = _rearr(np.stack([bout_c, b2], axis=1), 8)
        per_batch[b] = dict(
            wqk=wqk_r, wv=wv_r, wout=wout_r, w1=w1_r, w2=w2_r,
            qkb=_rearr(qkbias, 16), b1t=_rearr(b1_c, 32), bsum=bsum_t)

    in_maps = []
    for core in range(8):
        b, s = core // 4, core % 4
        blocks = [s, 7 - s]
        pb = per_batch[b]
        cols = np.concatenate([np.arange(bk * 128, (bk + 1) * 128) for bk in blocks])
        xTc = np.ascontiguousarray(x[b].T[:, cols])
        mrow = np.ascontiguousarray(m[b, cols, 0].reshape(1, CH))
        # KSC folds into v rows and the denominator ones-column (ratio exact)
        mqp = KSC * np.stack([m[b, bk * 128:(bk + 1) * 128, 0] for bk in blocks],
                             axis=1).astype(np.float32)
        kmask = np.zeros((128, 1664), np.float32)
        for jb in range(4):                      # group 0: both iq halves
            for iq, qb in enumerate(blocks):
                if jb < qb:
                    kmask[:, jb * 256 + iq * 128: jb * 256 + (iq + 1) * 128] = 1.0
        for u in range(3):                       # group 1: iq1-only jb 4..6
            if 4 + u < blocks[1]:
                kmask[:, 1024 + u * 128: 1024 + (u + 1) * 128] = 1.0
        for iq in range(2):                      # group 1: diag causal triangles
            kmask[:, 1408 + iq * 128: 1408 + (iq + 1) * 128] = causal01
        import ml_dtypes
        in_maps.append({
            "xT": xTc, "wqk": pb["wqk"], "wv": pb["wv"], "wout": pb["wout"],
            "w1": pb["w1"], "w2": pb["w2"], "qkb": pb["qkb"], "b1t": pb["b1t"],
            "bsum": pb["bsum"], "kmask": kmask.astype(ml_dtypes.bfloat16),
            "mqp": mqp, "mrow": mrow,
            "onescol": onescol, "kbias": kbias,
        })
    return in_maps


def kernel(**inputs):
    nc = get_nc()
    in_maps = make_in_maps(**inputs)
    res = run_bass_kernel_spmd(nc, in_maps, core_ids=list(range(8)))
    out = np.zeros((2, T, D), np.float32)
    for core in range(8):
        b, s = core // 4, core % 4
        o = res.results[core]["out_xT"]          # [D, CH]
        for iq, bk in enumerate([s, 7 - s]):
            out[b, bk * 128:(bk + 1) * 128, :] = o[:, iq * 128:(iq + 1) * 128].T
    return np.ascontiguousarray(out)

